# revision 65
# baseline (speedup 1.0000x reference)
import sys

for _p in ("/opt/trn_rl_repo",):
    if _p not in sys.path:
        sys.path.insert(0, _p)

import numpy as np

import concourse.bass as bass
import concourse.bacc as bacc
import concourse.mybir as mybir
from concourse.tile import TileContext
from concourse.bass_utils import run_bass_kernel_spmd

F32 = mybir.dt.float32
F16 = mybir.dt.float16
GE = mybir.AluOpType.is_ge
EQ = mybir.AluOpType.is_equal
ADD = mybir.AluOpType.add
MULT = mybir.AluOpType.mult
MAX = mybir.AluOpType.max
SUB = mybir.AluOpType.subtract

B, N, C, H, W = 4, 4, 256, 100, 152
HH = 50                  # output rows per core (H split in halves)
WP = W + 2               # padded pitch
RB = 10                  # output rows per block
NBLK = HH // RB
REG = RB * WP            # 1540 output-region elements per block
SRC_ROWS = RB + 2
SRC_LEN = SRC_ROWS * WP  # 1848 source elements per block (with halo rows)
PB = SRC_LEN + 4         # block tile width (over-read slack)
BASE_C = WP + 1          # offset of output (0,0) center in the block source
XL = (HH + 2) * WP + 4   # 8012 padded source length per half-channel row
YL = HH * WP             # 7700
SCALE = float(9 * C)     # fold 1/(9C) avg divide into the compare
SHIFTS = [di * WP + dj for di in range(3) for dj in range(3)]
USE_DMA_EQ = True   # census equality via SWDGE accumulate DMA + extract
DVE_XOR_K = (0, 2, 3, 5, 6, 8)  # shifts whose bit-extract runs on DVE (rest on Act)

_NC_CACHE = {}


def _chunks(total, step=512):
    out = []
    c0 = 0
    while c0 < total:
        out.append((c0, min(step, total - c0)))
        c0 += step
    return out


def build_nc():
    nc = bacc.Bacc(trn_type="TRN2")
    x1_h = nc.dram_tensor("x1", [128, 2, XL], F16, kind="ExternalInput")
    x2_h = nc.dram_tensor("x2", [N, 128, 2, XL], F16, kind="ExternalInput")
    wft_h = nc.dram_tensor("wft", [2, 2, 128, 128], F16, kind="ExternalInput")
    sel4_h = nc.dram_tensor("sel4", [4, 4, 128], F16, kind="ExternalInput")
    sel45_h = nc.dram_tensor("sel45", [45, 5, 128], F16, kind="ExternalInput")
    bf_h = nc.dram_tensor("bf", [2, 1, 128], F16, kind="ExternalInput")
    y_h = nc.dram_tensor("y", [2, 128, YL], F32, kind="ExternalOutput")

    with TileContext(nc) as tc:
        with (
            tc.tile_pool(name="const", bufs=1) as cpool,
            tc.tile_pool(name="pin1", bufs=2) as p1pool,
            tc.tile_pool(name="pin2", bufs=2) as p2pool,
            tc.tile_pool(name="srow", bufs=1) as srowpool,
            tc.tile_pool(name="s9", bufs=1) as s9pool,
            tc.tile_pool(name="urep", bufs=2) as ureppool,
            tc.tile_pool(name="wrep", bufs=1) as wreppool,
            tc.tile_pool(name="sig", bufs=2) as sigpool,
            tc.tile_pool(name="soft", bufs=1) as softpool,
            tc.tile_pool(name="wrow", bufs=1) as wrowpool,
            tc.tile_pool(name="fus", bufs=1) as fuspool,
            tc.tile_pool(name="psch", bufs=2, space="PSUM") as psch,
            tc.tile_pool(name="pssim", bufs=1, space="PSUM") as pssim,
            tc.tile_pool(name="psrc", bufs=2, space="PSUM") as psrc,
        ):
            eye5 = cpool.tile([128, 5, 5], F16, tag="eye5")
            nc.vector.memset(eye5[:], 0.0)
            for t5 in range(5):
                nc.vector.memset(eye5[:, t5, t5:t5 + 1], 1.0)
            eye4 = cpool.tile([128, 4, 4], F16, tag="eye4")
            nc.vector.memset(eye4[:], 0.0)
            for n in range(4):
                nc.vector.memset(eye4[:, n, n:n + 1], 1.0)
            sel4 = cpool.tile([4, 4, 128], F16, tag="sel4")
            nc.sync.dma_start(out=sel4[:], in_=sel4_h[:, :, :])
            sel45 = cpool.tile([45, 5, 128], F16, tag="sel45")
            nc.sync.dma_start(out=sel45[:], in_=sel45_h[:, :, :])
            neg1 = cpool.tile([128, 1], F32, tag="neg1")
            nc.vector.memset(neg1[:], -1.0)
            ones9 = cpool.tile([9, 128], F16, tag="ones9")
            nc.vector.memset(ones9[:], 1.0)
            ones_row = cpool.tile([1, 512], F16, tag="ones_row")
            nc.vector.memset(ones_row[:], 1.0)
            wft = {}
            for cc in range(2):
                for oc in range(2):
                    t = cpool.tile([128, 128], F16, tag=f"wft{cc}{oc}")
                    nc.sync.dma_start(out=t[:], in_=wft_h[cc, oc])
                    wft[(cc, oc)] = t
            bft = {}
            for oc in range(2):
                t = cpool.tile([1, 128], F16, tag=f"bf{oc}")
                nc.sync.dma_start(out=t[:], in_=bf_h[oc])
                bft[oc] = t

            def load_block(blk):
                off = blk * REG
                p1 = p1pool.tile([128, 2, PB], F16, tag="p1", name="p1")
                nc.sync.dma_start(out=p1[:, :, :], in_=x1_h[:, :, off:off + PB])
                p2 = []
                for n in range(N):
                    t = p2pool.tile([128, 2, PB], F16, tag=f"p2_{n}", name=f"p2_{n}")
                    nc.sync.dma_start(
                        out=t[:, :, :], in_=x2_h[n, :, :, off:off + PB]
                    )
                    p2.append(t)
                return p1, p2

            def prep_block(p1, p2):
                """Channel sums -> (via s9 gather) box-sum broadcast reps."""
                srcs = [p1] + p2
                srow5 = srowpool.tile([5, SRC_LEN + 8], F16, tag="srow5",
                                      name="srow5")
                nc.vector.memset(srow5[:, SRC_LEN:SRC_LEN + 8], 0.0)
                for c0, cl in _chunks(SRC_LEN):
                    ps = psch.tile([5, 512], F32, tag="psch", name="psch")
                    for t5, src in enumerate(srcs):
                        for h in range(2):
                            nc.tensor.matmul(
                                out=ps[:, 0:cl],
                                lhsT=eye5[:, t5, :],
                                rhs=src[:, h, c0:c0 + cl],
                                start=(t5 == 0 and h == 0),
                                stop=(t5 == 4 and h == 1),
                            )
                    nc.scalar.copy(out=srow5[:, c0:c0 + cl], in_=ps[:, 0:cl])
                # all 45 (tap, tensor) gather rows in one tile: 9 DMAs, then
                # per-t5 row selection happens inside the rep matmul's lhsT
                s9a = s9pool.tile([45, REG + 4], F16, tag="s9a", name="s9a")
                for di in range(3):
                    for dj in range(3):
                        off = di * WP + dj
                        r0 = (3 * di + dj) * 5
                        nc.sync.dma_start(
                            out=s9a[r0:r0 + 5, :],
                            in_=srow5[0:5, off:off + REG + 4],
                        )
                ureps = []
                for t5 in range(5):
                    ur = ureppool.tile([128, 1, REG], F16, tag=f"urep{t5}", name=f"urep{t5}")
                    for c0, cl in _chunks(REG):
                        ps = psrc.tile([128, 512], F32, tag="psrc", name="psrc")
                        nc.tensor.matmul(
                            out=ps[:, 0:cl],
                            lhsT=sel45[:, t5, :],
                            rhs=s9a[:, c0:c0 + cl],
                            start=True,
                            stop=True,
                        )
                        nc.scalar.mul(out=ur[:, 0, c0:c0 + cl], in_=ps[:, 0:cl],
                                      mul=1.0 / SCALE)
                    ureps.append(ur)
                return ureps

            def prep_block_fast(p1, p2):
                srcs = [p1] + p2
                ureps = []
                for t5, srctile in enumerate(srcs):
                    srow1 = srowpool.tile([1, SRC_LEN + 8], F16,
                                          tag="srow5", name=f"srow1_{t5}")
                    nc.vector.memset(srow1[:, SRC_LEN:SRC_LEN + 8], 0.0)
                    for c0, cl in _chunks(SRC_LEN):
                        ps = psch.tile([5, 512], F32, tag="psch", name="psch")
                        for h in range(2):
                            nc.tensor.matmul(
                                out=ps[:, 0:cl],
                                lhsT=eye5[:, 0, :],
                                rhs=srctile[:, h, c0:c0 + cl],
                                start=(h == 0),
                                stop=(h == 1),
                            )
                        nc.scalar.copy(out=srow1[:, c0:c0 + cl], in_=ps[0:1, 0:cl])
                    s9 = s9pool.tile([9, REG + 4], F16, tag="s9a",
                                     name=f"s9f_{t5}")
                    for di in range(3):
                        for dj in range(3):
                            off = di * WP + dj
                            nc.sync.dma_start(
                                out=s9[3 * di + dj:3 * di + dj + 1, :],
                                in_=srow1[0:1, off:off + REG + 4],
                            )
                    ur = ureppool.tile([128, 1, REG], F16, tag=f"urep{t5}",
                                       name=f"urep{t5}")
                    for c0, cl in _chunks(REG):
                        ps = psrc.tile([128, 512], F32, tag="psrc", name="psrc")
                        nc.tensor.matmul(
                            out=ps[:, 0:cl],
                            lhsT=ones9[:],
                            rhs=s9[:, c0:c0 + cl],
                            start=True,
                            stop=True,
                        )
                        nc.scalar.mul(out=ur[:, 0, c0:c0 + cl], in_=ps[:, 0:cl],
                                      mul=1.0 / SCALE)
                    ureps.append(ur)
                return ureps

            def census_shifts(p1, p2, ureps, ps_sim, ks):
                for k in ks:
                    bs = SHIFTS[k]
                    sg1 = sigpool.tile([128, 2, REG + 4], F16, tag="sg1",
                                       name="sg1", bufs=3)
                    nc.vector.tensor_tensor(
                        out=sg1[:, :, 0:REG],
                        in0=p1[:, :, bs:bs + REG],
                        in1=ureps[0][:, :, :].to_broadcast((128, 2, REG)),
                        op=GE,
                    )
                    # pass 1: all GEs (DVE) with the accumulate DMA queued right
                    # behind each; pass 2 extracts bits after the DMA latency
                    # has been hidden behind the later GEs.
                    sg2s = []
                    for n in range(N):
                        sg2 = sigpool.tile([128, 2, REG + 4], F16, tag="sg2",
                                           name="sg2", bufs=6)
                        for h in range(2):
                            nc.vector.tensor_tensor(
                                out=sg2[:, h, 0:REG],
                                in0=p2[n][:, h, bs:bs + REG],
                                in1=ureps[1 + n][:, 0, :],
                                op=GE,
                            )
                        # v = sg1 + sg2 via SWDGE accumulate DMA (per half:
                        # CCE accum silently truncates descriptors much beyond
                        # ~3KB, so keep each partition run at 3080B)
                        if USE_DMA_EQ:
                            for h in range(2):
                                nc.gpsimd.dma_start(
                                    out=sg2[:, h, 0:REG], in_=sg1[:, h, 0:REG],
                                    accum_op=ADD,
                                )
                        sg2s.append(sg2)
                    for n in range(N):
                        sg2 = sg2s[n]
                        if not USE_DMA_EQ:
                            nc.vector.tensor_tensor(
                                out=sg2[:, :, 0:REG], in0=sg1[:, :, 0:REG],
                                in1=sg2[:, :, 0:REG], op=EQ,
                            )
                        elif n < 2:
                            # DVE: -(v == 1) = negated XOR bit, so the n 0/1
                            # rows accumulate sim - 9C directly (no logit fix)
                            for h in range(2):
                                nc.vector.tensor_scalar(
                                    out=sg2[:, h, 0:REG], in0=sg2[:, h, 0:REG],
                                    scalar1=1.0, scalar2=-1.0, op0=EQ, op1=MULT,
                                )
                        else:
                            # Act: (v - 1)^2 -> match bit
                            for h in range(2):
                                nc.scalar.activation(
                                    out=sg2[:, h, 0:REG], in_=sg2[:, h, 0:REG],
                                    func=mybir.ActivationFunctionType.Square,
                                    bias=neg1[:],
                                )
                        for h in range(2):
                            for c0, cl in _chunks(REG):
                                nc.tensor.matmul(
                                    out=ps_sim[:, c0:c0 + cl],
                                    lhsT=eye4[:, n, :],
                                    rhs=sg2[:, h, c0:c0 + cl],
                                    start=(k == 0 and n == 0 and h == 0),
                                    stop=(k == 8 and n == 3 and h == 1),
                                )

            def softmax_wreps(ps_sim):
                sim4 = softpool.tile([4, REG], F32, tag="sim4", name="sim4", bufs=1)
                nc.scalar.copy(out=sim4[:, :], in_=ps_sim[:, :])
                st = [softpool.tile([RB, WP], F32, tag=f"st{n}", name=f"st{n}")
                      for n in range(N)]
                for n in range(N):
                    nc.scalar.dma_start(out=st[n][:, :], in_=sim4[n:n + 1, :])
                # n 2/3 rows hold match sums S (sim = S): logit = S - 9C to
                # align with the negated-XOR rows (logit = -X = sim - 9C)
                if USE_DMA_EQ:
                    for n in (2, 3):
                        nc.vector.tensor_scalar(
                            out=st[n][:], in0=st[n][:],
                            scalar1=-SCALE, scalar2=None, op0=ADD,
                        )
                m1 = softpool.tile([RB, WP], F32, tag="m1", name="m1")
                m2 = softpool.tile([RB, WP], F32, tag="m2", name="m2")
                nc.vector.tensor_tensor(
                    out=m1[:], in0=st[0][:], in1=st[1][:], op=MAX
                )
                nc.vector.tensor_tensor(
                    out=m2[:], in0=st[2][:], in1=st[3][:], op=MAX
                )
                nc.vector.tensor_tensor(out=m1[:], in0=m1[:], in1=m2[:], op=MAX)
                es = [softpool.tile([RB, WP], F32, tag=f"es{n}", name=f"es{n}")
                      for n in range(N)]
                for n in range(N):
                    nc.vector.tensor_tensor(
                        out=es[n][:], in0=st[n][:], in1=m1[:], op=SUB
                    )
                    nc.scalar.activation(
                        out=es[n][:], in_=es[n][:],
                        func=mybir.ActivationFunctionType.Exp,
                    )
                den = softpool.tile([RB, WP], F32, tag="den", name="den")
                nc.vector.tensor_tensor(
                    out=den[:], in0=es[0][:], in1=es[1][:], op=ADD
                )
                nc.vector.tensor_tensor(
                    out=den[:], in0=den[:], in1=es[2][:], op=ADD
                )
                nc.vector.tensor_tensor(
                    out=den[:], in0=den[:], in1=es[3][:], op=ADD
                )
                rec = softpool.tile([RB, WP], F32, tag="rec", name="rec")
                nc.vector.reciprocal(out=rec[:], in_=den[:])
                wrow4 = wrowpool.tile([4, REG], F16, tag="wrow4", name="wrow4")
                for n in range(N):
                    wq = softpool.tile([RB, WP], F16, tag=f"wq{n}", name=f"wq{n}")
                    nc.vector.tensor_tensor(
                        out=wq[:], in0=es[n][:], in1=rec[:], op=MULT
                    )
                    nc.scalar.dma_start(out=wrow4[n:n + 1, :], in_=wq[:, :])
                wreps = []
                for n in range(N):
                    wr = wreppool.tile([128, 1, REG], F16, tag=f"wrep{n}", name=f"wrep{n}")
                    for c0, cl in _chunks(REG):
                        ps = psrc.tile([128, 512], F32, tag="psrc", name="psrc")
                        nc.tensor.matmul(
                            out=ps[:, 0:cl],
                            lhsT=sel4[:, n, :],
                            rhs=wrow4[:, c0:c0 + cl],
                            start=True,
                            stop=True,
                        )
                        nc.scalar.copy(out=wr[:, 0, c0:c0 + cl], in_=ps[:, 0:cl])
                    wreps.append(wr)
                return wreps

            def fuse_conv(blk, p1, p2, wreps):
                offy = blk * REG
                # weighted nearby products in-place (p2 blocks are dead after)
                for n in range(N):
                    nc.vector.tensor_tensor(
                        out=p2[n][:, :, BASE_C:BASE_C + REG],
                        in0=wreps[n][:, :, :].to_broadcast((128, 2, REG)),
                        in1=p2[n][:, :, BASE_C:BASE_C + REG],
                        op=MULT,
                    )
                for oc in range(2):
                    for c0, cl in _chunks(REG):
                        ps = psrc.tile([128, 512], F32, tag="psrc", name="psrc")
                        first = True
                        for cc in range(2):
                            nc.tensor.matmul(
                                out=ps[:, 0:cl],
                                lhsT=wft[(cc, oc)][:],
                                rhs=p1[:, cc, BASE_C + c0:BASE_C + c0 + cl],
                                start=first,
                                stop=False,
                            )
                            first = False
                            for n in range(N):
                                nc.tensor.matmul(
                                    out=ps[:, 0:cl],
                                    lhsT=wft[(cc, oc)][:],
                                    rhs=p2[n][:, cc, BASE_C + c0:BASE_C + c0 + cl],
                                    start=False,
                                    stop=False,
                                )
                        nc.tensor.matmul(
                            out=ps[:, 0:cl],
                            lhsT=bft[oc][:],
                            rhs=ones_row[0:1, 0:cl],
                            start=False,
                            stop=True,
                        )
                        yo = fuspool.tile([128, 512], F32, tag="yo", name="yo", bufs=2)
                        nc.scalar.copy(out=yo[:, 0:cl], in_=ps[:, 0:cl])
                        nc.sync.dma_start(
                            out=y_h[oc, :, offy + c0:offy + c0 + cl],
                            in_=yo[:, 0:cl],
                        )

            def tail_block(blk, p1, p2, ps_sim):
                wreps = softmax_wreps(ps_sim)
                fuse_conv(blk, p1, p2, wreps)

            # software pipeline: prep(blk+1) is issued before census(blk) so
            # PE/Act prep work hides under the DVE-bound census; block k's
            # tail (softmax/fuse/conv) is emitted after the first shifts of
            # block k+1's census so DVE has work during the tail latency
            p1c, p2c = load_block(0)
            urc = prep_block(p1c, p2c)
            prev = None
            for blk in range(NBLK):
                if prev is not None:
                    tail_block(*prev)
                ps_sim = pssim.tile([4, REG], F32, tag="pssim", name="ps_sim")
                if blk + 1 < NBLK:
                    p1n, p2n = load_block(blk + 1)
                    urn = prep_block(p1n, p2n)
                else:
                    p1n = p2n = urn = None
                census_shifts(p1c, p2c, urc, ps_sim, range(0, 9))
                prev = (blk, p1c, p2c, ps_sim)
                p1c, p2c, urc = p1n, p2n, urn
            tail_block(*prev)
    nc.compile()
    return nc


def get_nc():
    if "nc" not in _NC_CACHE:
        _NC_CACHE["nc"] = build_nc()
    return _NC_CACHE["nc"]


def shard_inputs(features, nearby_features, w_fuse, b_fuse):
    features = np.asarray(features, np.float32)
    nearby_features = np.asarray(nearby_features, np.float32)
    wt = np.ascontiguousarray(np.asarray(w_fuse, np.float32).T).astype(np.float16)
    wft = np.zeros((2, 2, 128, 128), np.float16)
    for cc in range(2):
        for oc in range(2):
            wft[cc, oc] = wt[cc * 128:(cc + 1) * 128, oc * 128:(oc + 1) * 128]
    bf = np.zeros((2, 1, 128), np.float16)
    bq = np.asarray(b_fuse, np.float32).astype(np.float16)
    bf[0, 0] = bq[0:128]
    bf[1, 0] = bq[128:256]
    sel4 = np.zeros((4, 4, 128), np.float16)
    for n in range(4):
        sel4[n, n, :] = 1.0
    sel45 = np.zeros((45, 5, 128), np.float16)
    for tap in range(9):
        for t5 in range(5):
            sel45[tap * 5 + t5, t5, :] = 1.0
    cidx = np.clip(np.arange(-1, W + 1), 0, W - 1)
    in_maps = []
    for b in range(B):
        for half in range(2):
            h0 = half * HH
            ridx = np.clip(np.arange(h0 - 1, h0 + HH + 1), 0, H - 1)
            x1p = features[b][:, ridx][:, :, cidx].astype(np.float16).reshape(C, -1)
            x1 = np.zeros((128, 2, XL), np.float16)
            x1[:, 0, :x1p.shape[1]] = x1p[:128]
            x1[:, 1, :x1p.shape[1]] = x1p[128:]
            x2p = nearby_features[b][:, :, ridx][:, :, :, cidx].astype(
                np.float16).reshape(N, C, -1)
            x2 = np.zeros((N, 128, 2, XL), np.float16)
            x2[:, :, 0, :x2p.shape[2]] = x2p[:, :128]
            x2[:, :, 1, :x2p.shape[2]] = x2p[:, 128:]
            in_maps.append(
                {
                    "x1": np.ascontiguousarray(x1),
                    "x2": np.ascontiguousarray(x2),
                    "wft": wft,
                    "bf": bf,
                    "sel4": sel4,
                    "sel45": sel45,
                }
            )
    return in_maps


def gather_output(results):
    out = np.empty((B, C, H, W), np.float32)
    for i, r in enumerate(results):
        b, half = i // 2, i % 2
        y = np.asarray(r["y"]).reshape(2, 128, HH, WP)[:, :, :, :W]
        out[b, :, half * HH:(half + 1) * HH, :] = y.reshape(C, HH, W)
    return out


def kernel(features, nearby_features, w_fuse, b_fuse, _trace=False, _trace_kwargs=None):
    in_maps = shard_inputs(features, nearby_features, w_fuse, b_fuse)
    nc = get_nc()
    kw = {}
    if _trace:
        kw = dict(trace=True, **(_trace_kwargs or {}))
    res = run_bass_kernel_spmd(nc, in_maps, core_ids=list(range(8)), **kw)
    out = gather_output(res.results)
    kernel._last_result = res
    return out


# revision 66
# speedup vs baseline: 1.0103x; 1.0103x over previous
import sys

for _p in ("/opt/trn_rl_repo",):
    if _p not in sys.path:
        sys.path.insert(0, _p)

import numpy as np

import concourse.bass as bass
import concourse.bacc as bacc
import concourse.mybir as mybir
from concourse.tile import TileContext
from concourse.bass_utils import run_bass_kernel_spmd

F32 = mybir.dt.float32
F16 = mybir.dt.float16
GE = mybir.AluOpType.is_ge
EQ = mybir.AluOpType.is_equal
ADD = mybir.AluOpType.add
MULT = mybir.AluOpType.mult
MAX = mybir.AluOpType.max
SUB = mybir.AluOpType.subtract

B, N, C, H, W = 4, 4, 256, 100, 152
HH = 50                  # output rows per core (H split in halves)
WP = W + 2               # padded pitch
RB = 10                  # output rows per block
NBLK = HH // RB
REG = RB * WP            # 1540 output-region elements per block
SRC_ROWS = RB + 2
SRC_LEN = SRC_ROWS * WP  # 1848 source elements per block (with halo rows)
PB = SRC_LEN + 4         # block tile width (over-read slack)
BASE_C = WP + 1          # offset of output (0,0) center in the block source
XL = (HH + 2) * WP + 4   # 8012 padded source length per half-channel row
YL = HH * WP             # 7700
SCALE = float(9 * C)     # fold 1/(9C) avg divide into the compare
SHIFTS = [di * WP + dj for di in range(3) for dj in range(3)]
USE_DMA_EQ = True   # census equality via SWDGE accumulate DMA + extract
DVE_XOR_K = (0, 2, 3, 5, 6, 8)  # shifts whose bit-extract runs on DVE (rest on Act)

_NC_CACHE = {}


def _chunks(total, step=512):
    out = []
    c0 = 0
    while c0 < total:
        out.append((c0, min(step, total - c0)))
        c0 += step
    return out


def build_nc():
    nc = bacc.Bacc(trn_type="TRN2")
    x1_h = nc.dram_tensor("x1", [128, 2, XL], F16, kind="ExternalInput")
    x2_h = nc.dram_tensor("x2", [N, 128, 2, XL], F16, kind="ExternalInput")
    wft_h = nc.dram_tensor("wft", [2, 2, 128, 128], F16, kind="ExternalInput")
    sel4_h = nc.dram_tensor("sel4", [4, 4, 128], F16, kind="ExternalInput")
    sel45_h = nc.dram_tensor("sel45", [45, 5, 128], F16, kind="ExternalInput")
    bf_h = nc.dram_tensor("bf", [2, 1, 128], F16, kind="ExternalInput")
    y_h = nc.dram_tensor("y", [2, 128, YL], F32, kind="ExternalOutput")

    with TileContext(nc) as tc:
        with (
            tc.tile_pool(name="const", bufs=1) as cpool,
            tc.tile_pool(name="pin1", bufs=2) as p1pool,
            tc.tile_pool(name="pin2", bufs=2) as p2pool,
            tc.tile_pool(name="srow", bufs=1) as srowpool,
            tc.tile_pool(name="s9", bufs=1) as s9pool,
            tc.tile_pool(name="urep", bufs=2) as ureppool,
            tc.tile_pool(name="wrep", bufs=1) as wreppool,
            tc.tile_pool(name="sig", bufs=2) as sigpool,
            tc.tile_pool(name="soft", bufs=1) as softpool,
            tc.tile_pool(name="wrow", bufs=1) as wrowpool,
            tc.tile_pool(name="fus", bufs=1) as fuspool,
            tc.tile_pool(name="psch", bufs=2, space="PSUM") as psch,
            tc.tile_pool(name="pssim", bufs=1, space="PSUM") as pssim,
            tc.tile_pool(name="psrc", bufs=2, space="PSUM") as psrc,
        ):
            eye5 = cpool.tile([128, 5, 5], F16, tag="eye5")
            nc.vector.memset(eye5[:], 0.0)
            for t5 in range(5):
                nc.vector.memset(eye5[:, t5, t5:t5 + 1], 1.0)
            eye4 = cpool.tile([128, 4, 4], F16, tag="eye4")
            nc.vector.memset(eye4[:], 0.0)
            for n in range(4):
                nc.vector.memset(eye4[:, n, n:n + 1], 1.0)
            sel4 = cpool.tile([4, 4, 128], F16, tag="sel4")
            nc.sync.dma_start(out=sel4[:], in_=sel4_h[:, :, :])
            sel45 = cpool.tile([45, 5, 128], F16, tag="sel45")
            nc.sync.dma_start(out=sel45[:], in_=sel45_h[:, :, :])
            neg1 = cpool.tile([128, 1], F32, tag="neg1")
            nc.vector.memset(neg1[:], -1.0)
            ones9 = cpool.tile([9, 128], F16, tag="ones9")
            nc.vector.memset(ones9[:], 1.0)
            ones_row = cpool.tile([1, 512], F16, tag="ones_row")
            nc.vector.memset(ones_row[:], 1.0)
            wft = {}
            for cc in range(2):
                for oc in range(2):
                    t = cpool.tile([128, 128], F16, tag=f"wft{cc}{oc}")
                    nc.sync.dma_start(out=t[:], in_=wft_h[cc, oc])
                    wft[(cc, oc)] = t
            bft = {}
            for oc in range(2):
                t = cpool.tile([1, 128], F16, tag=f"bf{oc}")
                nc.sync.dma_start(out=t[:], in_=bf_h[oc])
                bft[oc] = t

            def load_block(blk):
                off = blk * REG
                p1 = p1pool.tile([128, 2, PB], F16, tag="p1", name="p1")
                nc.sync.dma_start(out=p1[:, :, :], in_=x1_h[:, :, off:off + PB])
                p2 = []
                for n in range(N):
                    t = p2pool.tile([128, 2, PB], F16, tag=f"p2_{n}", name=f"p2_{n}")
                    nc.sync.dma_start(
                        out=t[:, :, :], in_=x2_h[n, :, :, off:off + PB]
                    )
                    p2.append(t)
                return p1, p2

            def prep_block(p1, p2):
                """Channel sums -> (via s9 gather) box-sum broadcast reps."""
                srcs = [p1] + p2
                srow5 = srowpool.tile([5, SRC_LEN + 8], F16, tag="srow5",
                                      name="srow5")
                nc.vector.memset(srow5[:, SRC_LEN:SRC_LEN + 8], 0.0)
                for c0, cl in _chunks(SRC_LEN):
                    ps = psch.tile([5, 512], F32, tag="psch", name="psch")
                    for t5, src in enumerate(srcs):
                        for h in range(2):
                            nc.tensor.matmul(
                                out=ps[:, 0:cl],
                                lhsT=eye5[:, t5, :],
                                rhs=src[:, h, c0:c0 + cl],
                                start=(t5 == 0 and h == 0),
                                stop=(t5 == 4 and h == 1),
                            )
                    nc.scalar.copy(out=srow5[:, c0:c0 + cl], in_=ps[:, 0:cl])
                # all 45 (tap, tensor) gather rows in one tile: 9 DMAs, then
                # per-t5 row selection happens inside the rep matmul's lhsT
                s9a = s9pool.tile([45, REG + 4], F16, tag="s9a", name="s9a")
                for di in range(3):
                    for dj in range(3):
                        off = di * WP + dj
                        r0 = (3 * di + dj) * 5
                        nc.sync.dma_start(
                            out=s9a[r0:r0 + 5, :],
                            in_=srow5[0:5, off:off + REG + 4],
                        )
                ureps = []
                for t5 in range(5):
                    ur = ureppool.tile([128, 1, REG], F16, tag=f"urep{t5}", name=f"urep{t5}")
                    for c0, cl in _chunks(REG):
                        ps = psrc.tile([128, 512], F32, tag="psrc", name="psrc")
                        nc.tensor.matmul(
                            out=ps[:, 0:cl],
                            lhsT=sel45[:, t5, :],
                            rhs=s9a[:, c0:c0 + cl],
                            start=True,
                            stop=True,
                        )
                        nc.scalar.mul(out=ur[:, 0, c0:c0 + cl], in_=ps[:, 0:cl],
                                      mul=1.0 / SCALE)
                    ureps.append(ur)
                return ureps

            def prep_block_fast(p1, p2):
                srcs = [p1] + p2
                ureps = []
                for t5, srctile in enumerate(srcs):
                    srow1 = srowpool.tile([1, SRC_LEN + 8], F16,
                                          tag="srow5", name=f"srow1_{t5}")
                    nc.vector.memset(srow1[:, SRC_LEN:SRC_LEN + 8], 0.0)
                    for c0, cl in _chunks(SRC_LEN):
                        ps = psch.tile([5, 512], F32, tag="psch", name="psch")
                        for h in range(2):
                            nc.tensor.matmul(
                                out=ps[:, 0:cl],
                                lhsT=eye5[:, 0, :],
                                rhs=srctile[:, h, c0:c0 + cl],
                                start=(h == 0),
                                stop=(h == 1),
                            )
                        nc.scalar.copy(out=srow1[:, c0:c0 + cl], in_=ps[0:1, 0:cl])
                    s9 = s9pool.tile([9, REG + 4], F16, tag="s9a",
                                     name=f"s9f_{t5}")
                    for di in range(3):
                        for dj in range(3):
                            off = di * WP + dj
                            nc.sync.dma_start(
                                out=s9[3 * di + dj:3 * di + dj + 1, :],
                                in_=srow1[0:1, off:off + REG + 4],
                            )
                    ur = ureppool.tile([128, 1, REG], F16, tag=f"urep{t5}",
                                       name=f"urep{t5}")
                    for c0, cl in _chunks(REG):
                        ps = psrc.tile([128, 512], F32, tag="psrc", name="psrc")
                        nc.tensor.matmul(
                            out=ps[:, 0:cl],
                            lhsT=ones9[:],
                            rhs=s9[:, c0:c0 + cl],
                            start=True,
                            stop=True,
                        )
                        nc.scalar.mul(out=ur[:, 0, c0:c0 + cl], in_=ps[:, 0:cl],
                                      mul=1.0 / SCALE)
                    ureps.append(ur)
                return ureps

            def census_shifts(p1, p2, ureps, ps_sim, ks):
                for k in ks:
                    bs = SHIFTS[k]
                    sg1 = sigpool.tile([128, 2, REG + 4], F16, tag="sg1",
                                       name="sg1", bufs=3)
                    for h in range(2):
                        nc.vector.tensor_tensor(
                            out=sg1[:, h, 0:REG],
                            in0=p1[:, h, bs:bs + REG],
                            in1=ureps[0][:, 0, :],
                            op=GE,
                        )
                    # pass 1: all GEs (DVE) with the accumulate DMA queued right
                    # behind each; pass 2 extracts bits after the DMA latency
                    # has been hidden behind the later GEs.
                    sg2s = []
                    for n in range(N):
                        sg2 = sigpool.tile([128, 2, REG + 4], F16, tag="sg2",
                                           name="sg2", bufs=6)
                        for h in range(2):
                            nc.vector.tensor_tensor(
                                out=sg2[:, h, 0:REG],
                                in0=p2[n][:, h, bs:bs + REG],
                                in1=ureps[1 + n][:, 0, :],
                                op=GE,
                            )
                        # v = sg1 + sg2 via SWDGE accumulate DMA (per half:
                        # CCE accum silently truncates descriptors much beyond
                        # ~3KB, so keep each partition run at 3080B)
                        if USE_DMA_EQ:
                            for h in range(2):
                                nc.gpsimd.dma_start(
                                    out=sg2[:, h, 0:REG], in_=sg1[:, h, 0:REG],
                                    accum_op=ADD,
                                )
                        sg2s.append(sg2)
                    for n in range(N):
                        sg2 = sg2s[n]
                        if not USE_DMA_EQ:
                            nc.vector.tensor_tensor(
                                out=sg2[:, :, 0:REG], in0=sg1[:, :, 0:REG],
                                in1=sg2[:, :, 0:REG], op=EQ,
                            )
                        elif n < 2:
                            # DVE: -(v == 1) = negated XOR bit, so the n 0/1
                            # rows accumulate sim - 9C directly (no logit fix)
                            for h in range(2):
                                nc.vector.tensor_scalar(
                                    out=sg2[:, h, 0:REG], in0=sg2[:, h, 0:REG],
                                    scalar1=1.0, scalar2=-1.0, op0=EQ, op1=MULT,
                                )
                        else:
                            # Act: (v - 1)^2 -> match bit
                            for h in range(2):
                                nc.scalar.activation(
                                    out=sg2[:, h, 0:REG], in_=sg2[:, h, 0:REG],
                                    func=mybir.ActivationFunctionType.Square,
                                    bias=neg1[:],
                                )
                        for h in range(2):
                            for c0, cl in _chunks(REG):
                                nc.tensor.matmul(
                                    out=ps_sim[:, c0:c0 + cl],
                                    lhsT=eye4[:, n, :],
                                    rhs=sg2[:, h, c0:c0 + cl],
                                    start=(k == 0 and n == 0 and h == 0),
                                    stop=(k == 8 and n == 3 and h == 1),
                                )

            def softmax_wreps(ps_sim):
                sim4 = softpool.tile([4, REG], F32, tag="sim4", name="sim4", bufs=1)
                nc.scalar.copy(out=sim4[:, :], in_=ps_sim[:, :])
                st = [softpool.tile([RB, WP], F32, tag=f"st{n}", name=f"st{n}")
                      for n in range(N)]
                for n in range(N):
                    nc.scalar.dma_start(out=st[n][:, :], in_=sim4[n:n + 1, :])
                # n 2/3 rows hold match sums S (sim = S): logit = S - 9C to
                # align with the negated-XOR rows (logit = -X = sim - 9C)
                if USE_DMA_EQ:
                    for n in (2, 3):
                        nc.vector.tensor_scalar(
                            out=st[n][:], in0=st[n][:],
                            scalar1=-SCALE, scalar2=None, op0=ADD,
                        )
                m1 = softpool.tile([RB, WP], F32, tag="m1", name="m1")
                m2 = softpool.tile([RB, WP], F32, tag="m2", name="m2")
                nc.vector.tensor_tensor(
                    out=m1[:], in0=st[0][:], in1=st[1][:], op=MAX
                )
                nc.vector.tensor_tensor(
                    out=m2[:], in0=st[2][:], in1=st[3][:], op=MAX
                )
                nc.vector.tensor_tensor(out=m1[:], in0=m1[:], in1=m2[:], op=MAX)
                es = [softpool.tile([RB, WP], F32, tag=f"es{n}", name=f"es{n}")
                      for n in range(N)]
                for n in range(N):
                    nc.vector.tensor_tensor(
                        out=es[n][:], in0=st[n][:], in1=m1[:], op=SUB
                    )
                    nc.scalar.activation(
                        out=es[n][:], in_=es[n][:],
                        func=mybir.ActivationFunctionType.Exp,
                    )
                den = softpool.tile([RB, WP], F32, tag="den", name="den")
                nc.vector.tensor_tensor(
                    out=den[:], in0=es[0][:], in1=es[1][:], op=ADD
                )
                nc.vector.tensor_tensor(
                    out=den[:], in0=den[:], in1=es[2][:], op=ADD
                )
                nc.vector.tensor_tensor(
                    out=den[:], in0=den[:], in1=es[3][:], op=ADD
                )
                rec = softpool.tile([RB, WP], F32, tag="rec", name="rec")
                nc.vector.reciprocal(out=rec[:], in_=den[:])
                wrow4 = wrowpool.tile([4, REG], F16, tag="wrow4", name="wrow4")
                for n in range(N):
                    wq = softpool.tile([RB, WP], F16, tag=f"wq{n}", name=f"wq{n}")
                    nc.vector.tensor_tensor(
                        out=wq[:], in0=es[n][:], in1=rec[:], op=MULT
                    )
                    nc.scalar.dma_start(out=wrow4[n:n + 1, :], in_=wq[:, :])
                wreps = []
                for n in range(N):
                    wr = wreppool.tile([128, 1, REG], F16, tag=f"wrep{n}", name=f"wrep{n}")
                    for c0, cl in _chunks(REG):
                        ps = psrc.tile([128, 512], F32, tag="psrc", name="psrc")
                        nc.tensor.matmul(
                            out=ps[:, 0:cl],
                            lhsT=sel4[:, n, :],
                            rhs=wrow4[:, c0:c0 + cl],
                            start=True,
                            stop=True,
                        )
                        nc.scalar.copy(out=wr[:, 0, c0:c0 + cl], in_=ps[:, 0:cl])
                    wreps.append(wr)
                return wreps

            def fuse_conv(blk, p1, p2, wreps):
                offy = blk * REG
                # weighted nearby products in-place (p2 blocks are dead after)
                for n in range(N):
                    nc.vector.tensor_tensor(
                        out=p2[n][:, :, BASE_C:BASE_C + REG],
                        in0=wreps[n][:, :, :].to_broadcast((128, 2, REG)),
                        in1=p2[n][:, :, BASE_C:BASE_C + REG],
                        op=MULT,
                    )
                for oc in range(2):
                    for c0, cl in _chunks(REG):
                        ps = psrc.tile([128, 512], F32, tag="psrc", name="psrc")
                        first = True
                        for cc in range(2):
                            nc.tensor.matmul(
                                out=ps[:, 0:cl],
                                lhsT=wft[(cc, oc)][:],
                                rhs=p1[:, cc, BASE_C + c0:BASE_C + c0 + cl],
                                start=first,
                                stop=False,
                            )
                            first = False
                            for n in range(N):
                                nc.tensor.matmul(
                                    out=ps[:, 0:cl],
                                    lhsT=wft[(cc, oc)][:],
                                    rhs=p2[n][:, cc, BASE_C + c0:BASE_C + c0 + cl],
                                    start=False,
                                    stop=False,
                                )
                        nc.tensor.matmul(
                            out=ps[:, 0:cl],
                            lhsT=bft[oc][:],
                            rhs=ones_row[0:1, 0:cl],
                            start=False,
                            stop=True,
                        )
                        yo = fuspool.tile([128, 512], F32, tag="yo", name="yo", bufs=2)
                        nc.scalar.copy(out=yo[:, 0:cl], in_=ps[:, 0:cl])
                        nc.sync.dma_start(
                            out=y_h[oc, :, offy + c0:offy + c0 + cl],
                            in_=yo[:, 0:cl],
                        )

            def tail_block(blk, p1, p2, ps_sim):
                wreps = softmax_wreps(ps_sim)
                fuse_conv(blk, p1, p2, wreps)

            # software pipeline: prep(blk+1) is issued before census(blk) so
            # PE/Act prep work hides under the DVE-bound census; block k's
            # tail (softmax/fuse/conv) is emitted after the first shifts of
            # block k+1's census so DVE has work during the tail latency
            p1c, p2c = load_block(0)
            urc = prep_block(p1c, p2c)
            prev = None
            for blk in range(NBLK):
                if prev is not None:
                    tail_block(*prev)
                ps_sim = pssim.tile([4, REG], F32, tag="pssim", name="ps_sim")
                if blk + 1 < NBLK:
                    p1n, p2n = load_block(blk + 1)
                    urn = prep_block(p1n, p2n)
                else:
                    p1n = p2n = urn = None
                census_shifts(p1c, p2c, urc, ps_sim, range(0, 9))
                prev = (blk, p1c, p2c, ps_sim)
                p1c, p2c, urc = p1n, p2n, urn
            tail_block(*prev)
    nc.compile()
    return nc


def get_nc():
    if "nc" not in _NC_CACHE:
        _NC_CACHE["nc"] = build_nc()
    return _NC_CACHE["nc"]


def shard_inputs(features, nearby_features, w_fuse, b_fuse):
    features = np.asarray(features, np.float32)
    nearby_features = np.asarray(nearby_features, np.float32)
    wt = np.ascontiguousarray(np.asarray(w_fuse, np.float32).T).astype(np.float16)
    wft = np.zeros((2, 2, 128, 128), np.float16)
    for cc in range(2):
        for oc in range(2):
            wft[cc, oc] = wt[cc * 128:(cc + 1) * 128, oc * 128:(oc + 1) * 128]
    bf = np.zeros((2, 1, 128), np.float16)
    bq = np.asarray(b_fuse, np.float32).astype(np.float16)
    bf[0, 0] = bq[0:128]
    bf[1, 0] = bq[128:256]
    sel4 = np.zeros((4, 4, 128), np.float16)
    for n in range(4):
        sel4[n, n, :] = 1.0
    sel45 = np.zeros((45, 5, 128), np.float16)
    for tap in range(9):
        for t5 in range(5):
            sel45[tap * 5 + t5, t5, :] = 1.0
    cidx = np.clip(np.arange(-1, W + 1), 0, W - 1)
    in_maps = []
    for b in range(B):
        for half in range(2):
            h0 = half * HH
            ridx = np.clip(np.arange(h0 - 1, h0 + HH + 1), 0, H - 1)
            x1p = features[b][:, ridx][:, :, cidx].astype(np.float16).reshape(C, -1)
            x1 = np.zeros((128, 2, XL), np.float16)
            x1[:, 0, :x1p.shape[1]] = x1p[:128]
            x1[:, 1, :x1p.shape[1]] = x1p[128:]
            x2p = nearby_features[b][:, :, ridx][:, :, :, cidx].astype(
                np.float16).reshape(N, C, -1)
            x2 = np.zeros((N, 128, 2, XL), np.float16)
            x2[:, :, 0, :x2p.shape[2]] = x2p[:, :128]
            x2[:, :, 1, :x2p.shape[2]] = x2p[:, 128:]
            in_maps.append(
                {
                    "x1": np.ascontiguousarray(x1),
                    "x2": np.ascontiguousarray(x2),
                    "wft": wft,
                    "bf": bf,
                    "sel4": sel4,
                    "sel45": sel45,
                }
            )
    return in_maps


def gather_output(results):
    out = np.empty((B, C, H, W), np.float32)
    for i, r in enumerate(results):
        b, half = i // 2, i % 2
        y = np.asarray(r["y"]).reshape(2, 128, HH, WP)[:, :, :, :W]
        out[b, :, half * HH:(half + 1) * HH, :] = y.reshape(C, HH, W)
    return out


def kernel(features, nearby_features, w_fuse, b_fuse, _trace=False, _trace_kwargs=None):
    in_maps = shard_inputs(features, nearby_features, w_fuse, b_fuse)
    nc = get_nc()
    kw = {}
    if _trace:
        kw = dict(trace=True, **(_trace_kwargs or {}))
    res = run_bass_kernel_spmd(nc, in_maps, core_ids=list(range(8)), **kw)
    out = gather_output(res.results)
    kernel._last_result = res
    return out
